# revision 3
# baseline (speedup 1.0000x reference)
"""Trainium2 Bass kernel: 4096x4096 valid cross-correlation with an 11x11
filter + scalar bias, sharded column-wise across 8 NeuronCores.

Strategy
--------
Host-side sharding (halo = overlapping column slices, no collectives):
core c gets input columns [512c, 512c + 522) (core 7 shifted left to stay
in bounds) and produces output columns [512c, 512c + 512).

Per-core compute: conv expressed as banded matmuls on the TensorEngine.
For each kernel column dj, a banded stationary matrix
    B_dj[k, m] = w[k - m, dj]   (0 <= k - m < 11)
contracts over 128 image rows, while column-shifted slices of the image
slab stream as the moving operand:
    out[m, n] += sum_k B_dj[k, m] * x[r0 + k, n + dj]
Accumulating the 11 dj-shifted matmuls in one PSUM bank yields the full
11x11 correlation for a [118, 512] output tile.

Column-split beats row-split because the 4086-row extent divides into
118-row slabs with ~1% waste (35 slabs) instead of the 13% waste of
512-row shards (5 slabs incl. a 40-row tail that still streams full
width). Operands are bf16 (same 1 PE row/cycle as float32r, half the
DMA/SBUF; quantization rel err ~2.4e-3 « 2e-2 gate). PSUM accumulation
stays fp32.
"""

import os
import sys

import numpy as np

for _p in ("/opt/trn_rl_repo", "/root/.axon_site/_ro/trn_rl_repo"):
    if os.path.isdir(_p) and _p not in sys.path:
        sys.path.insert(0, _p)

# The device run goes through jax's axon PJRT backend; make sure it is
# visible if jax has not been initialized yet.
_jp = os.environ.get("JAX_PLATFORMS", "")
if "axon" not in _jp.split(","):
    os.environ["JAX_PLATFORMS"] = ("axon," + _jp).strip(",")

import ml_dtypes

import concourse.bacc as bacc
import concourse.bass as bass
import concourse.mybir as mybir
import concourse.tile as tile
from concourse.bass_utils import run_bass_kernel_spmd

H = W = 4096
KH = KW = 11
OH = OW = H - KH + 1  # 4086
NCORES = 8
COLS_OUT = 512            # output columns per core
COLS_IN = COLS_OUT + KW - 1  # 522
M_FULL = 118              # output rows per full slab (contraction K = 128)
# (x row offset, out row offset, M out rows, band column offset) per slab.
# 34 full slabs + a 74-row tail that reads the last 128 image rows and
# picks the shifted band columns 44..117.
SLABS = [(118 * s, 118 * s, 118, 0) for s in range(34)]
SLABS.append((H - 128, 34 * 118, OH - 34 * 118, 128 - (H - 34 * 118)))
assert SLABS[-1] == (3968, 4012, 74, 44)

_cache: dict = {}
LAST_RESULT = None  # BassKernelResults of the most recent device run


def _build():
    f32 = mybir.dt.float32
    bf16 = mybir.dt.bfloat16
    nc = bacc.Bacc("TRN2", target_bir_lowering=False, debug=False,
                   num_devices=NCORES)
    xs_d = nc.dram_tensor("xs", [H, COLS_IN], bf16, kind="ExternalInput")
    bd_d = nc.dram_tensor("bands", [128, KW * M_FULL], bf16,
                          kind="ExternalInput")
    bias_d = nc.dram_tensor("biasv", [1, 1], f32, kind="ExternalInput")
    out_d = nc.dram_tensor("out", [OH, COLS_OUT], f32, kind="ExternalOutput")

    with tile.TileContext(nc) as tc:
        with (
            tc.tile_pool(name="bp", bufs=1) as bp,
            tc.tile_pool(name="xp", bufs=1) as xp,
            tc.tile_pool(name="op", bufs=4) as op,
            tc.tile_pool(name="pp", bufs=6, space=bass.MemorySpace.PSUM) as pp,
            tc.tile_pool(name="pw", bufs=1, space=bass.MemorySpace.PSUM) as pw,
        ):
            # bands first: the warmup matmuls read them
            bt = bp.tile([128, KW * M_FULL], bf16, name="bt")
            nc.sync.dma_start(bt[:], bd_d.ap()[:, :])

            # bias: one-packet DMA, then broadcast across partitions with a
            # K=1 matmul against a ones row
            bias_sb = bp.tile([1, 1], f32, name="bias_sb")
            nc.sync.dma_start(bias_sb[:], bias_d.ap()[:, :])
            ones_t = bp.tile([1, 128], f32, name="ones_t")
            nc.gpsimd.memset(ones_t[:], 1.0)

            # all slab loads issued upfront (35 x 133 KB = 4.7 MB, 36.5 KB
            # per partition in SBUF): the sync queue stalls once >8 DMAs are
            # in flight, but nothing else runs on it, so that's harmless.
            xts = []
            for si, (r0, _, _, _) in enumerate(SLABS):
                xt = xp.tile([128, COLS_IN], bf16, tag=f"xt{si}",
                             name=f"xt{si}")
                nc.sync.dma_start(xt[:], xs_d.ap()[r0:r0 + 128, :])
                xts.append(xt)

            # warm the PE's HAM clock gate (~3.4 us budget) while slab
            # loads are in flight; also covers the bands DMA latency.
            warm = pw.tile([118, 512], f32, name="warm")
            for i in range(7):
                nc.tensor.matmul(warm[:, :], bt[:, 0:118], bt[:, 0:512],
                                 start=(i == 0), stop=(i == 6))

            bias_ps = pw.tile([128, 1], f32, name="bias_ps")
            nc.tensor.matmul(bias_ps[:], ones_t[:], bias_sb[:],
                             start=True, stop=True)
            bias_bc = bp.tile([128, 1], f32, name="bias_bc")
            nc.scalar.copy(bias_bc[:], bias_ps[:])

            for si, (r0, o0, M, boff) in enumerate(SLABS):
                xt = xts[si]
                pt = pp.tile([M, 512], f32, tag="ps", name=f"ps{si}")
                for dj in range(KW):
                    nc.tensor.matmul(
                        pt[:, :],
                        bt[:, dj * M_FULL + boff: dj * M_FULL + boff + M],
                        xt[:, dj: dj + COLS_OUT],
                        start=(dj == 0),
                        stop=(dj == KW - 1),
                    )
                ot = op.tile([M, COLS_OUT], f32, tag="ot", name=f"ot{si}")
                nc.scalar.activation(
                    ot[:, :], pt[:, :],
                    mybir.ActivationFunctionType.Identity,
                    bias=bias_bc[0:M, :],
                )
                # store trigger on the scalar queue: follows this slab's ACT
                # in order, and keeps the sync queue free for loads
                nc.scalar.dma_start(out_d.ap()[o0:o0 + M, :], ot[:])
    nc.compile()
    return nc


def _bands_from_weight(weight: np.ndarray) -> np.ndarray:
    b = np.zeros((128, KW * M_FULL), np.float32)
    for dj in range(KW):
        col = weight[:, dj].astype(np.float32)
        for m in range(M_FULL):
            b[m:m + KH, dj * M_FULL + m] = col
    return b.astype(ml_dtypes.bfloat16)


def kernel(x: np.ndarray, weight: np.ndarray, bias: np.ndarray,
           _trace: bool = False, **_trace_kwargs) -> np.ndarray:
    global LAST_RESULT
    x = np.asarray(x, dtype=np.float32)
    weight = np.asarray(weight, dtype=np.float32)
    bias_v = np.asarray(bias, dtype=np.float32).reshape(1, 1)

    if "nc" not in _cache:
        _cache["nc"] = _build()
    nc = _cache["nc"]

    bands = _bands_from_weight(weight)
    xb = x.astype(ml_dtypes.bfloat16)
    starts = [min(c * COLS_OUT, W - COLS_IN) for c in range(NCORES)]
    in_maps = [
        {"xs": np.ascontiguousarray(xb[:, s:s + COLS_IN]),
         "bands": bands,
         "biasv": bias_v}
        for s in starts
    ]
    res = run_bass_kernel_spmd(nc, in_maps, core_ids=list(range(NCORES)),
                               trace=_trace, **_trace_kwargs)
    LAST_RESULT = res

    out = np.empty((OH, OW), dtype=np.float32)
    for c, s in enumerate(starts):
        r = res.results[c]["out"]
        g0 = c * COLS_OUT          # first global output col wanted from core c
        keep0 = g0 - s             # 0 for cores 0-6, 10 for core 7
        take = min(COLS_OUT - keep0, OW - g0)
        out[:, g0:g0 + take] = r[:, keep0:keep0 + take]
    return out


# revision 4
# speedup vs baseline: 1.0073x; 1.0073x over previous
"""Trainium2 Bass kernel: 4096x4096 valid cross-correlation with an 11x11
filter + scalar bias, sharded column-wise across 8 NeuronCores.

Strategy
--------
Host-side sharding (halo = overlapping column slices, no collectives):
core c gets input columns [512c, 512c + 522) (core 7 shifted left to stay
in bounds) and produces output columns [512c, 512c + 512).

Per-core compute: conv expressed as banded matmuls on the TensorEngine.
For each kernel column dj, a banded stationary matrix
    B_dj[k, m] = w[k - m, dj]   (0 <= k - m < 11)
contracts over 128 image rows, while column-shifted slices of the image
slab stream as the moving operand:
    out[m, n] += sum_k B_dj[k, m] * x[r0 + k, n + dj]
Accumulating the 11 dj-shifted matmuls in one PSUM bank yields the full
11x11 correlation for a [118, 512] output tile.

Column-split beats row-split because the 4086-row extent divides into
118-row slabs with ~1% waste (35 slabs) instead of the 13% waste of
512-row shards (5 slabs incl. a 40-row tail that still streams full
width). Operands are bf16 (same 1 PE row/cycle as float32r, half the
DMA/SBUF; quantization rel err ~2.4e-3 « 2e-2 gate). PSUM accumulation
stays fp32.
"""

import os
import sys

import numpy as np

for _p in ("/opt/trn_rl_repo", "/root/.axon_site/_ro/trn_rl_repo"):
    if os.path.isdir(_p) and _p not in sys.path:
        sys.path.insert(0, _p)

# The device run goes through jax's axon PJRT backend; make sure it is
# visible if jax has not been initialized yet.
_jp = os.environ.get("JAX_PLATFORMS", "")
if "axon" not in _jp.split(","):
    os.environ["JAX_PLATFORMS"] = ("axon," + _jp).strip(",")

import ml_dtypes

import concourse.bacc as bacc
import concourse.bass as bass
import concourse.mybir as mybir
import concourse.tile as tile
from concourse.bass_utils import run_bass_kernel_spmd

H = W = 4096
KH = KW = 11
OH = OW = H - KH + 1  # 4086
NCORES = 8
COLS_OUT = 512            # output columns per core
COLS_IN = COLS_OUT + KW - 1  # 522
M_FULL = 118              # output rows per full slab (contraction K = 128)
# (x row offset, out row offset, M out rows, band column offset) per slab.
# 34 full slabs + a 74-row tail that reads the last 128 image rows and
# picks the shifted band columns 44..117.
SLABS = [(118 * s, 118 * s, 118, 0) for s in range(34)]
SLABS.append((H - 128, 34 * 118, OH - 34 * 118, 128 - (H - 34 * 118)))
assert SLABS[-1] == (3968, 4012, 74, 44)

_cache: dict = {}
LAST_RESULT = None  # BassKernelResults of the most recent device run


def _build():
    f32 = mybir.dt.float32
    bf16 = mybir.dt.bfloat16
    nc = bacc.Bacc("TRN2", target_bir_lowering=False, debug=False,
                   num_devices=NCORES)
    xs_d = nc.dram_tensor("xs", [H, COLS_IN], bf16, kind="ExternalInput")
    bd_d = nc.dram_tensor("bands", [128, KW * M_FULL], bf16,
                          kind="ExternalInput")
    bias_d = nc.dram_tensor("biasv", [1, 1], f32, kind="ExternalInput")
    out_d = nc.dram_tensor("out", [OH, COLS_OUT], f32, kind="ExternalOutput")

    with tile.TileContext(nc) as tc:
        with (
            tc.tile_pool(name="bp", bufs=1) as bp,
            tc.tile_pool(name="xp", bufs=1) as xp,
            tc.tile_pool(name="op", bufs=4) as op,
            tc.tile_pool(name="pp", bufs=6, space=bass.MemorySpace.PSUM) as pp,
            tc.tile_pool(name="pw", bufs=1, space=bass.MemorySpace.PSUM) as pw,
        ):
            # bands + bias DMAs first on the sync queue
            bt = bp.tile([128, KW * M_FULL], bf16, name="bt")
            nc.sync.dma_start(bt[:], bd_d.ap()[:, :])
            bias_sb = bp.tile([1, 1], f32, name="bias_sb")
            nc.sync.dma_start(bias_sb[:], bias_d.ap()[:, :])
            ones_t = bp.tile([1, 128], f32, name="ones_t")
            nc.gpsimd.memset(ones_t[:], 1.0)

            # warm tile is memset, not DMA'd: the PE clock-gate warmup can
            # start immediately, overlapping the bands/slab loads.
            wsrc = bp.tile([128, 512], bf16, name="wsrc")
            nc.gpsimd.memset(wsrc[:], 1.0)

            PREFETCH = 8
            xts = []
            for si, (r0, _, _, _) in enumerate(SLABS):
                xt = xp.tile([128, COLS_IN], bf16, tag=f"xt{si}",
                             name=f"xt{si}")
                xts.append(xt)
                if si < PREFETCH:
                    nc.sync.dma_start(xt[:], xs_d.ap()[r0:r0 + 128, :])

            # warm the PE's HAM clock gate (~3.4 us budget) while the
            # first slab loads are in flight
            warm = pw.tile([128, 512], f32, name="warm")
            for i in range(8):
                nc.tensor.matmul(warm[:, :], wsrc[:, 0:128], wsrc[:, :],
                                 start=(i == 0), stop=(i == 7))

            # bias broadcast across partitions with a K=1 matmul (a
            # 128-packet broadcast DMA is ~13us; this chain is ~2us)
            bias_ps = pw.tile([128, 1], f32, name="bias_ps")
            nc.tensor.matmul(bias_ps[:], ones_t[:], bias_sb[:],
                             start=True, stop=True)
            bias_bc = bp.tile([128, 1], f32, name="bias_bc")
            nc.scalar.copy(bias_bc[:], bias_ps[:])

            for si, (r0, o0, M, boff) in enumerate(SLABS):
                xt = xts[si]
                pt = pp.tile([M, 512], f32, tag="ps", name=f"ps{si}")
                for dj in range(KW):
                    nc.tensor.matmul(
                        pt[:, :],
                        bt[:, dj * M_FULL + boff: dj * M_FULL + boff + M],
                        xt[:, dj: dj + COLS_OUT],
                        start=(dj == 0),
                        stop=(dj == KW - 1),
                    )
                ot = op.tile([M, COLS_OUT], f32, tag="ot", name=f"ot{si}")
                nc.scalar.activation(
                    ot[:, :], pt[:, :],
                    mybir.ActivationFunctionType.Identity,
                    bias=bias_bc[0:M, :],
                )
                # store on sync: sync-triggered DMAs (Q_I) fan out across
                # all 16 SDMA engines; scalar-triggered ones (Q_X) land on
                # only 2 and became the bottleneck. The store trigger waits
                # on this slab's ACT, which also paces the next prefetch
                # load trigger behind it to an 8-slab lead.
                nc.sync.dma_start(out_d.ap()[o0:o0 + M, :], ot[:])
                if si + PREFETCH < len(SLABS):
                    r0n = SLABS[si + PREFETCH][0]
                    xtn = xts[si + PREFETCH]
                    nc.sync.dma_start(xtn[:], xs_d.ap()[r0n:r0n + 128, :])
    nc.compile()
    return nc


def _bands_from_weight(weight: np.ndarray) -> np.ndarray:
    b = np.zeros((128, KW * M_FULL), np.float32)
    for dj in range(KW):
        col = weight[:, dj].astype(np.float32)
        for m in range(M_FULL):
            b[m:m + KH, dj * M_FULL + m] = col
    return b.astype(ml_dtypes.bfloat16)


def kernel(x: np.ndarray, weight: np.ndarray, bias: np.ndarray,
           _trace: bool = False, **_trace_kwargs) -> np.ndarray:
    global LAST_RESULT
    x = np.asarray(x, dtype=np.float32)
    weight = np.asarray(weight, dtype=np.float32)
    bias_v = np.asarray(bias, dtype=np.float32).reshape(1, 1)

    if "nc" not in _cache:
        _cache["nc"] = _build()
    nc = _cache["nc"]

    bands = _bands_from_weight(weight)
    xb = x.astype(ml_dtypes.bfloat16)
    starts = [min(c * COLS_OUT, W - COLS_IN) for c in range(NCORES)]
    in_maps = [
        {"xs": np.ascontiguousarray(xb[:, s:s + COLS_IN]),
         "bands": bands,
         "biasv": bias_v}
        for s in starts
    ]
    res = run_bass_kernel_spmd(nc, in_maps, core_ids=list(range(NCORES)),
                               trace=_trace, **_trace_kwargs)
    LAST_RESULT = res

    out = np.empty((OH, OW), dtype=np.float32)
    for c, s in enumerate(starts):
        r = res.results[c]["out"]
        g0 = c * COLS_OUT          # first global output col wanted from core c
        keep0 = g0 - s             # 0 for cores 0-6, 10 for core 7
        take = min(COLS_OUT - keep0, OW - g0)
        out[:, g0:g0 + take] = r[:, keep0:keep0 + take]
    return out
